# revision 22
# baseline (speedup 1.0000x reference)
"""Multi-head self-attention on 8 TRN2 NeuronCores.

Sharding: (batch, head-quad). Core c owns batch c//4 and heads
4*(c%4)..4*(c%4)+3, i.e. a 256-column slice of Wq/Wk/Wv and the matching
256-row slice of Wo. The host sums the 4 partial outputs per batch (the
tensor-parallel all-reduce) and adds the output bias.

Structure (v3): the attention inner loop runs on 512-query chunks and
processes a HEAD PAIR per segment so the two score matmuls (contraction
dk=64) land on different PE row-groups (tile_position (0,0)/(64,0)) and
execute concurrently — the packed pair costs one matmul slot instead of
two. The exp of both heads' scores is a single 1024-wide activation from
one 2-bank PSUM tile; ACT paces the loop and all projection / output
work drips into the PE slack through a dedicated 2-slot PSUM chain ring,
so the score ring never loses lookahead.

- K projection runs d-outer across 8 PSUM banks to pipeline with X DMA;
  V' for key tiles 0-7 reuses those banks d-outer right after; V' tiles
  8-15 drip as chains inside segment 0.
- Q/K biases are folded into the PSUM->SBUF drain (per-partition bias on
  ACT / tensor_scalar_add on DVE) instead of K=1 matmuls.  V' keeps the
  ones-row matmul because its bias varies along the free dim (and it
  also provides the softmax-normalizer column).
- Dummy LDWEIGHTS fill PE idle slots: the HAM activity monitor
  re-throttles the whole core to half clock when the PE duty cycle sags
  (costs ~20% wall clock), so staying busy is faster than idling.
- softmax 1/z is broadcast across partitions via a DRAM bounce
  (partition-stride-0 DMA read); gpsimd Q7 compute also downclocks.
- one dma_start per weight matrix (descriptor building on the queue
  engines is ~0.6us each and delays the X stream otherwise).

Precision: f16 activations/weights on the PE; PSUM f32; partial outputs
f16 summed in f64 on host.
"""

import numpy as np

B, S, D, H, DK = 2, 2048, 1024, 16, 64
NCORES = 8
GPB = 4                    # head-groups per batch
HPC = H // GPB             # heads per core = 4
COLS = HPC * DK            # feature columns per core = 256
VCOLS = HPC * (DK + 1)     # V' columns incl per-head ones = 260
ND = D // 128              # 8 contraction chunks
NT = S // 512              # 4 token chunks (free dim 512)
NKT = S // 128             # 16 key tiles
NQC = S // 512             # 4 query chunks
NPR = 2                    # head pairs per core
NE = D // 128              # 8 output-column chunks
NVA = 8                    # V' tiles projected d-outer before attention

_CACHE = {}


def _build_program():
    from contextlib import ExitStack

    import concourse.bacc as bacc
    import concourse.bass as bass
    import concourse.mybir as mybir
    import concourse.tile as tile

    f32 = mybir.dt.float32
    f16 = mybir.dt.float16
    Exp = mybir.ActivationFunctionType.Exp
    Mult = mybir.AluOpType.mult

    nc = bacc.Bacc("TRN2", target_bir_lowering=False, debug=False,
                   num_devices=NCORES)

    XT = nc.dram_tensor("xt", [D, S], f16, kind="ExternalInput").ap()
    WQ = nc.dram_tensor("wq", [D + 1, COLS], f16, kind="ExternalInput").ap()
    WK = nc.dram_tensor("wk", [D + 1, COLS], f16, kind="ExternalInput").ap()
    WV = nc.dram_tensor("wv", [D + 1, VCOLS], f16, kind="ExternalInput").ap()
    WO = nc.dram_tensor("wo", [COLS, D], f16, kind="ExternalInput").ap()
    OT = nc.dram_tensor("ot", [D, S], f16, kind="ExternalOutput").ap()

    def packed_dma(dst_tile, src, rows_per_chunk, chunk_w, nchunk):
        """One DMA: DRAM [nchunk*rows, chunk_w] -> SBUF [rows, nchunk*chunk_w]."""
        dst_ap = bass.AP(tensor=dst_tile.tensor, offset=dst_tile.offset,
                         ap=[dst_tile.ap[0], [chunk_w, nchunk], [1, chunk_w]])
        src_ap = bass.AP(tensor=src.tensor, offset=src.offset,
                         ap=[[chunk_w, rows_per_chunk],
                             [rows_per_chunk * chunk_w, nchunk], [1, chunk_w]])
        nc.sync.dma_start(dst_ap, src_ap)

    with tile.TileContext(nc) as tc, ExitStack() as ctx:
        consts = ctx.enter_context(tc.tile_pool(name="consts", bufs=1))
        drp = ctx.enter_context(tc.tile_pool(name="drp", bufs=2, space="DRAM"))

        # ---- weights (one DMA each; wk first — K proj is critical
        # path) and X chunks on the gpsimd queue in parallel ----
        # dummy weight-load source for PE warmup/fillers
        dummy_w = consts.tile([128, 128], f16, name="dummy_w")
        nc.vector.memset(dummy_w, 0.0)

        def filler(n):
            """Dummy weight loads: keep the PE activity monitor from
            re-throttling the core during PE-idle stretches (the core
            drops to half clock otherwise, ~20% wall)."""
            for _ in range(n):
                nc.tensor.ldweights(dummy_w)

        # warm the PE while the launch-phase DMA queues boot (~9us):
        # the HAM starts cold (1.2GHz) and needs ~3.4us of activity
        filler(52)

        wk_all = consts.tile([128, ND * COLS], f16, name="wk_all")
        packed_dma(wk_all, WK, 128, COLS, ND)

        xc = []
        for d in range(ND):
            t = consts.tile([128, S], f16, name=f"xc_{d}")
            eng = nc.sync if d < 4 else nc.gpsimd
            eng.dma_start(t, XT[d * 128:(d + 1) * 128, :])
            xc.append(t)

        wv_all = consts.tile([128, ND * VCOLS], f16, name="wv_all")
        packed_dma(wv_all, WV, 128, VCOLS, ND)
        wq_all = consts.tile([128, ND * COLS], f16, name="wq_all")
        packed_dma(wq_all, WQ, 128, COLS, ND)
        wo_all = consts.tile([128, 2 * D], f16, name="wo_all")
        packed_dma(wo_all, WO, 128, D, 2)

        # per-partition bias columns for the K/Q drains
        def bias_col(name, src_row, ct):
            t16 = consts.tile([128, 1], f16, name=name + "h")
            dst = bass.AP(tensor=t16.tensor, offset=t16.offset,
                          ap=[t16.ap[0], [1, 1]])
            s = src_row[0:1, ct * 128:(ct + 1) * 128]
            src_ap = bass.AP(tensor=s.tensor, offset=s.offset,
                             ap=[[1, 128], [1, 1]])
            nc.sync.dma_start(dst, src_ap)
            t = consts.tile([128, 1], f32, name=name)
            nc.vector.tensor_copy(t, t16)
            return t

        wkb = [bias_col(f"wkb_{ct}", WK[D:D + 1, :], ct) for ct in range(2)]
        wqb = [bias_col(f"wqb_{ct}", WQ[D:D + 1, :], ct) for ct in range(2)]
        wv_b = consts.tile([1, VCOLS], f16, name="wv_b")
        nc.sync.dma_start(wv_b, WV[D:D + 1, :])

        ones_row = consts.tile([1, S], f16, name="ones_row")
        nc.vector.memset(ones_row, 1.0)
        onecol_16 = consts.tile([1, DK], f16, name="onecol")
        nc.vector.memset(onecol_16, 1.0)

        KT = [consts.tile([128, S], f16, name=f"KT_{ct}") for ct in range(2)]
        QT = [consts.tile([128, S], f16, name=f"QT_{ct}") for ct in range(2)]
        VP = [consts.tile([128, VCOLS], f16, name=f"VP_{kt}")
              for kt in range(NKT)]

        def wk_c(d, cs):
            return wk_all[:, d * COLS + cs.start:d * COLS + cs.stop]

        def wq_c(d, cs):
            return wq_all[:, d * COLS + cs.start:d * COLS + cs.stop]

        def wv_c(d):
            return wv_all[:, d * VCOLS:(d + 1) * VCOLS]

        # ---- phase 1: K projection d-outer (consumes X chunks as they
        # arrive), then V' tiles 0..NVA-1 d-outer on the same banks ----
        with tc.tile_pool(name="pk", bufs=8, space="PSUM") as pk:
            kps = [pk.tile([128, 512], f32, tag="k", name=f"kps_{i}")
                   for i in range(8)]
            for d in range(ND):
                for ct in range(2):
                    cs = slice(ct * 128, (ct + 1) * 128)
                    for tch in range(NT):
                        ts = slice(tch * 512, (tch + 1) * 512)
                        nc.tensor.matmul(kps[ct * NT + tch],
                                         lhsT=wk_c(d, cs), rhs=xc[d][:, ts],
                                         start=(d == 0), stop=(d == ND - 1))
            for ct in range(2):
                for tch in range(NT):
                    ts = slice(tch * 512, (tch + 1) * 512)
                    if (ct * NT + tch) % 2:
                        nc.scalar.activation(
                            KT[ct][:, ts], kps[ct * NT + tch],
                            mybir.ActivationFunctionType.Identity,
                            bias=wkb[ct])
                    else:
                        nc.vector.tensor_scalar_add(KT[ct][:, ts],
                                                    kps[ct * NT + tch],
                                                    wkb[ct])
            filler(12)
            vps = [pk.tile([128, VCOLS], f32, tag="k", name=f"vps_{i}")
                   for i in range(NVA)]
            for d in range(ND + 1):
                for kt in range(NVA):
                    ks = slice(kt * 128, (kt + 1) * 128)
                    if d < ND:
                        lhsT, rhs = xc[d][:, ks], wv_c(d)
                    else:
                        lhsT, rhs = ones_row[:, ks], wv_b
                    nc.tensor.matmul(vps[kt], lhsT=lhsT, rhs=rhs,
                                     start=(d == 0), stop=(d == ND))
            for kt in range(NVA):
                if kt % 2:
                    nc.scalar.copy(VP[kt], vps[kt])
                else:
                    nc.vector.tensor_copy(VP[kt], vps[kt])

        # ---- phase 2: flat attention + drip-fed projections ----
        with tc.tile_pool(name="ps", bufs=2, space="PSUM") as ps, \
                tc.tile_pool(name="expp", bufs=14) as expp, \
                tc.tile_pool(name="attnp", bufs=4) as attnp, \
                tc.tile_pool(name="zp", bufs=2) as zp, \
                tc.tile_pool(name="zbp", bufs=2) as zbp, \
                tc.tile_pool(name="obp", bufs=3) as obp:

            segs = [(qc, pr) for qc in range(NQC) for pr in range(NPR)]
            attn_t = {}

            def sc_pair(qc, pr, kt):
                """Packed score pair: head A on PE rows 0-63, head B on
                rows 64-127 — concurrent via row tiling."""
                qs = slice(qc * 512, (qc + 1) * 512)
                ks = slice(kt * 128, (kt + 1) * 128)
                sc = ps.tile([128, 1024], f32, tag="sc", bufs=2,
                             name=f"sc_{qc}_{pr}_{kt}")
                nc.tensor.matmul(sc[:, 0:512], lhsT=KT[pr][0:64, ks],
                                 rhs=QT[pr][0:64, qs], start=True, stop=True)
                nc.tensor.matmul(sc[:, 512:1024], lhsT=KT[pr][64:128, ks],
                                 rhs=QT[pr][64:128, qs], start=True,
                                 stop=True)
                return sc

            def q_chain_jobs(pr, tch):
                """Project one Q quadrant; 8 single-matmul jobs + a
                bias-folding drain, dripped one per kt. The PSUM chain
                slot is allocated lazily by the first job so the
                tag-ring rotation matches emission order."""
                cs = slice(pr * 128, (pr + 1) * 128)
                ts = slice(tch * 512, (tch + 1) * 512)
                st = {}

                def mk(d):
                    def job():
                        if d == 0:
                            st["p"] = ps.tile([128, 512], f32, tag="c",
                                              bufs=2, name=f"qch_{pr}_{tch}")
                        nc.tensor.matmul(st["p"], lhsT=wq_c(d, cs),
                                         rhs=xc[d][:, ts],
                                         start=(d == 0), stop=(d == ND - 1))
                    return job

                def cast():
                    nc.vector.tensor_scalar_add(QT[pr][:, ts], st["p"],
                                                wqb[pr])

                return [mk(d) for d in range(ND)] + [cast]

            def q_chain(pr, tch):
                for job in q_chain_jobs(pr, tch):
                    job()

            def vp_chain(kt, cast_eng="scalar"):
                """Project one V' token-tile into a chain slot."""
                ks = slice(kt * 128, (kt + 1) * 128)
                p = ps.tile([128, VCOLS], f32, tag="c", bufs=2,
                            name=f"vch_{kt}")
                for d in range(ND + 1):
                    if d < ND:
                        lhsT, rhs = xc[d][:, ks], wv_c(d)
                    else:
                        lhsT, rhs = ones_row[:, ks], wv_b
                    nc.tensor.matmul(p, lhsT=lhsT, rhs=rhs,
                                     start=(d == 0), stop=(d == ND))
                if cast_eng == "scalar":
                    nc.scalar.copy(VP[kt], p)
                else:
                    nc.vector.tensor_copy(VP[kt], p)

            def outproj_e(qc, e, tail=False):
                es = slice(e * 128, (e + 1) * 128)
                po = ps.tile([128, 512], f32, tag="c", bufs=2,
                             name=f"po_{qc}_{e}")
                for cc in range(2):
                    nc.tensor.matmul(
                        po, lhsT=wo_all[:, cc * D + es.start:cc * D + es.stop],
                        rhs=attn_t[(qc, cc)], start=(cc == 0), stop=(cc == 1))
                ob = obp.tile([128, 512], f16, tag="ob", name=f"ob_{qc}_{e}")
                if tail:
                    # ACT is idle at the tail — split the drain
                    nc.vector.tensor_copy(ob[:, 0:256], po[:, 0:256])
                    nc.scalar.copy(ob[:, 256:512], po[:, 256:512])
                else:
                    nc.vector.tensor_copy(ob, po)
                qbase = qc * 512
                eng = nc.sync if e % 2 == 0 else nc.gpsimd
                eng.dma_start(OT[es, qbase:qbase + 512], ob)

            def finish_seg(patt, qc, pr, tail):
                """Normalize the accumulated pair: z rows -> reciprocal
                -> partition broadcast -> multiply into attn_t."""
                at = attnp.tile([128, 512], f16, tag="attn",
                                name=f"attn_{qc}_{pr}")
                attn_t[(qc, pr)] = at
                zs = zp.tile([1, 1024], f32, tag="zs", name=f"zs_{qc}_{pr}")
                nc.vector.tensor_copy(zs, patt[DK:DK + 1, :])
                zr = zp.tile([1, 1024], f32, tag="z", name=f"zr_{qc}_{pr}")
                nc.vector.reciprocal_approx_fast(zr, zs)
                if tail:
                    filler(30)
                    # last segment: broadcast 1/z via PE outer products
                    # (short latency; the DRAM bounce would gate the tail)
                    zr16 = zp.tile([1, 1024], f16, tag="z16",
                                   name=f"zr16_{qc}_{pr}")
                    nc.vector.tensor_copy(zr16, zr)
                    araw = zbp.tile([DK, 1024], f32, tag="zb",
                                    name=f"araw_{qc}_{pr}")
                    nc.scalar.copy(araw, patt[0:DK, :])
                    zba = ps.tile([64, 512], f32, tag="c", bufs=2,
                                  name=f"zba_{qc}_{pr}")
                    nc.tensor.matmul(zba, lhsT=onecol_16[:, 0:DK],
                                     rhs=zr16[:, 0:512], start=True,
                                     stop=True)
                    zbb = ps.tile([64, 512], f32, tag="c", bufs=2,
                                  name=f"zbb_{qc}_{pr}")
                    nc.tensor.matmul(zbb, lhsT=onecol_16[:, 0:DK],
                                     rhs=zr16[:, 512:1024], start=True,
                                     stop=True)
                    nc.vector.tensor_tensor(at[0:DK, :], araw[:, 0:512],
                                            zba, Mult)
                    nc.vector.tensor_tensor(at[DK:128, :],
                                            araw[:, 512:1024], zbb, Mult)
                else:
                    # broadcast 1/z across partitions via a DRAM bounce
                    # (partition-stride-0 DMA read)
                    scratch = drp.tile([1, 1024], f32, tag="scr",
                                       name=f"scr_{qc}_{pr}")
                    nc.gpsimd.dma_start(scratch, zr)
                    zb = zbp.tile([DK, 1024], f32, tag="zb",
                                  name=f"zb_{qc}_{pr}")
                    row = scratch[0:1, :]
                    nc.gpsimd.dma_start(
                        zb,
                        bass.AP(tensor=row.tensor, offset=row.offset,
                                ap=[[0, DK]] + row.ap[1:]))
                    nc.vector.tensor_tensor(at[0:DK, :], patt[0:DK, 0:512],
                                            zb[:, 0:512], Mult)
                    nc.vector.tensor_tensor(at[DK:128, :],
                                            patt[0:DK, 512:1024],
                                            zb[:, 512:1024], Mult)
                filler(4)

            # drip plan: (si, kt) -> list of (cost_ns, thunk).  Each kt
            # is topped up with dummy weight-loads to a uniform PE load
            # slightly above the ACT exp pace: a deliberately PE-bound
            # loop stays at full clock (HAM) and tolerates jitter,
            # whereas a balanced one oscillates against the ring limits.
            drip = {}

            def add_drip(si, kt, job, cost=230):
                drip.setdefault((si, kt), []).append((cost, job))

            # s0: the p1 Q quadrant early (segment 1 needs it), then V'
            # JIT for tiles NVA..15 with a 4-kt lead
            qj = q_chain_jobs(1, 0)
            for j, job in enumerate(qj):
                add_drip(0, j * 3 // 9, job)       # kts 0..3
            for j, kt in enumerate(range(NVA, NKT)):
                add_drip(0, kt - 4, (lambda k, e: lambda: vp_chain(k, e))(
                    kt, "scalar" if j % 2 else "vector"), cost=1200)
            # q-chain segments: segment 1 -> tch 1, 3 -> tch 2, 5 -> tch 3
            for si, tch in ((1, 1), (3, 2), (5, 3)):
                for pr in range(2):
                    jobs = q_chain_jobs(pr, tch)
                    for j, job in enumerate(jobs):
                        add_drip(si, min(j + 7 * pr, NKT - 1), job)
            # outproj segments: qc finished after segment 2*qc+1; its 8
            # column chunks drip over the following segment (one per
            # odd kt)
            for qc in range(NQC - 1):
                for e in range(NE):
                    add_drip(2 * qc + 2, 2 * e + 1,
                             (lambda q, ee: lambda: outproj_e(q, ee))(qc, e),
                             cost=520)

            def make_attnv(patt, kt, ex, vs_a, vs_b):
                def emit():
                    nc.tensor.matmul(patt[:, 0:512], lhsT=VP[kt][:, vs_a],
                                     rhs=ex[:, 0:512], start=(kt == 0),
                                     stop=(kt == NKT - 1))
                    nc.tensor.matmul(patt[:, 512:1024],
                                     lhsT=VP[kt][:, vs_b],
                                     rhs=ex[:, 512:1024], start=(kt == 0),
                                     stop=(kt == NKT - 1))
                return emit

            # pre-loop: first Q quadrant, then the first score pair.
            # attnV runs one kt BEHIND the exp/score stream so the score
            # matmuls are never queued behind an exp-waiting attnV
            # (head-of-line blocking starves ACT and trips the HAM
            # throttle).
            q_chain(0, 0)
            sc = sc_pair(0, 0, 0)
            pend = None
            pend_fin = None
            for si, (qc, pr) in enumerate(segs):
                patt = ps.tile([DK + 1, 1024], f32, tag="acc", bufs=1,
                               name=f"patt_{qc}_{pr}")
                vs_a = slice((2 * pr) * (DK + 1), (2 * pr) * (DK + 1) + DK + 1)
                vs_b = slice((2 * pr + 1) * (DK + 1),
                             (2 * pr + 1) * (DK + 1) + DK + 1)
                for kt in range(NKT):
                    ex = expp.tile([128, 1024], f16, tag="e",
                                   name=f"ex_{qc}_{pr}_{kt}")
                    nc.scalar.activation(ex, sc, Exp, scale=0.125)
                    # prefetch the next score pair (crossing boundaries)
                    if kt + 1 < NKT:
                        sc = sc_pair(qc, pr, kt + 1)
                    elif si + 1 < len(segs):
                        sc = sc_pair(segs[si + 1][0], segs[si + 1][1], 0)
                    if pend is not None:
                        pend()
                    if pend_fin is not None:
                        # previous segment's last attnV just went out;
                        # its normalization must precede this patt's
                        # first write (the accumulator is 1-deep)
                        pend_fin()
                        pend_fin = None
                    cost = 0
                    for c, job in drip.pop((si, kt), ()):
                        job()
                        cost += c
                    filler(max(0, (460 - cost) // 110))
                    pend = make_attnv(patt, kt, ex, vs_a, vs_b)
                pend_fin = (lambda p, q, r: lambda: finish_seg(
                    p, q, r, tail=False))(patt, qc, pr)
            pend()
            finish_seg(patt, NQC - 1, NPR - 1, tail=True)
            # flush the last query chunk's output projection
            for e in range(NE):
                outproj_e(NQC - 1, e, tail=True)

    nc.compile()
    return nc


def _get_program():
    if "nc" not in _CACHE:
        _CACHE["nc"] = _build_program()
    return _CACHE["nc"]


def _install_ntff_hook():
    """Provide the antenv.axon_hooks shim this container's antenv lacks so
    run_bass_kernel_spmd(trace=True) can capture NTFF profiles."""
    import sys
    import types

    try:
        import antenv

        if hasattr(antenv, "axon_hooks"):
            return
        mod = types.ModuleType("antenv.axon_hooks")
        mod._hook = None
        mod.set_axon_ntff_profile_hook = lambda h: setattr(mod, "_hook", h)
        mod.get_axon_ntff_profile_hook = lambda: mod._hook
        sys.modules["antenv.axon_hooks"] = mod
        antenv.axon_hooks = mod
        from trn_agent_boot.trn_boot import _ntff_profile_via_ctypes

        mod.set_axon_ntff_profile_hook(
            _ntff_profile_via_ctypes("/opt/axon/libaxon_pjrt.so"))
    except Exception:
        pass


def kernel(X, Wq, bq, Wk, bk, Wv, bv, Wo, bo, _profile=False, _trace_cores=None):
    from concourse.bass_utils import run_bass_kernel_spmd

    if _profile:
        _install_ntff_hook()

    nc = _get_program()

    X = np.asarray(X, np.float32)
    Wq, Wk, Wv, Wo = (np.asarray(w, np.float32) for w in (Wq, Wk, Wv, Wo))
    bq, bk, bv, bo = (np.asarray(v, np.float32) for v in (bq, bk, bv, bo))

    in_maps = []
    for c in range(NCORES):
        b, g = c // GPB, c % GPB
        cs = slice(g * COLS, (g + 1) * COLS)
        wq_aug = np.vstack([Wq[:, cs], bq[cs][None, :]])
        wk_aug = np.vstack([Wk[:, cs], bk[cs][None, :]])
        wv_aug = np.zeros((D + 1, VCOLS), np.float32)
        for h in range(HPC):
            hc = slice(g * COLS + h * DK, g * COLS + (h + 1) * DK)
            wv_aug[:D, h * (DK + 1):h * (DK + 1) + DK] = Wv[:, hc]
            wv_aug[D, h * (DK + 1):h * (DK + 1) + DK] = bv[hc]
            wv_aug[D, h * (DK + 1) + DK] = 1.0
        in_maps.append({
            "xt": np.ascontiguousarray(X[b].T).astype(np.float16),
            "wq": wq_aug.astype(np.float16),
            "wk": wk_aug.astype(np.float16),
            "wv": wv_aug.astype(np.float16),
            "wo": np.ascontiguousarray(Wo[cs, :]).astype(np.float16),
        })

    res = run_bass_kernel_spmd(
        nc, in_maps, core_ids=list(range(NCORES)),
        trace=_profile,
        trace_cores=(_trace_cores if _trace_cores is not None
                     else ([0] if _profile else None)),
    )

    out = np.empty((B, S, D), np.float32)
    for b in range(B):
        ot = res.results[b * GPB]["ot"].astype(np.float64)
        for g in range(1, GPB):
            ot += res.results[b * GPB + g]["ot"]
        out[b] = (ot.T + bo).astype(np.float32)
    if _profile:
        kernel.last_exec_time_ns = res.exec_time_ns
        kernel.last_results = res
    return out


# revision 24
# speedup vs baseline: 1.0310x; 1.0310x over previous
"""Multi-head self-attention on 8 TRN2 NeuronCores.

Sharding: (batch, head-quad). Core c owns batch c//4 and heads
4*(c%4)..4*(c%4)+3, i.e. a 256-column slice of Wq/Wk/Wv and the matching
256-row slice of Wo. The host sums the 4 partial outputs per batch (the
tensor-parallel all-reduce) and adds the output bias.

Structure (v3): the attention inner loop runs on 512-query chunks and
processes a HEAD PAIR per segment so the two score matmuls (contraction
dk=64) land on different PE row-groups (tile_position (0,0)/(64,0)) and
execute concurrently — the packed pair costs one matmul slot instead of
two. The exp of both heads' scores is a single 1024-wide activation from
one 2-bank PSUM tile; ACT paces the loop and all projection / output
work drips into the PE slack through a dedicated 2-slot PSUM chain ring,
so the score ring never loses lookahead.

- K projection runs d-outer across 8 PSUM banks to pipeline with X DMA;
  V' for key tiles 0-7 reuses those banks d-outer right after; V' tiles
  8-15 drip as chains inside segment 0.
- Q/K biases are folded into the PSUM->SBUF drain (per-partition bias on
  ACT / tensor_scalar_add on DVE) instead of K=1 matmuls.  V' keeps the
  ones-row matmul because its bias varies along the free dim (and it
  also provides the softmax-normalizer column).
- Dummy LDWEIGHTS fill PE idle slots: the HAM activity monitor
  re-throttles the whole core to half clock when the PE duty cycle sags
  (costs ~20% wall clock), so staying busy is faster than idling.
- softmax 1/z is broadcast across partitions via a DRAM bounce
  (partition-stride-0 DMA read); gpsimd Q7 compute also downclocks.
- one dma_start per weight matrix (descriptor building on the queue
  engines is ~0.6us each and delays the X stream otherwise).

Precision: f16 activations/weights on the PE; PSUM f32; partial outputs
f16 summed in f64 on host.
"""

import numpy as np

B, S, D, H, DK = 2, 2048, 1024, 16, 64
NCORES = 8
GPB = 4                    # head-groups per batch
HPC = H // GPB             # heads per core = 4
COLS = HPC * DK            # feature columns per core = 256
VCOLS = HPC * (DK + 1)     # V' columns incl per-head ones = 260
ND = D // 128              # 8 contraction chunks
NT = S // 512              # 4 token chunks (free dim 512)
NKT = S // 128             # 16 key tiles
NQC = S // 512             # 4 query chunks
NPR = 2                    # head pairs per core
NE = D // 128              # 8 output-column chunks
NVA = 8                    # V' tiles projected d-outer before attention

_CACHE = {}


def _build_program():
    from contextlib import ExitStack

    import concourse.bacc as bacc
    import concourse.bass as bass
    import concourse.mybir as mybir
    import concourse.tile as tile

    f32 = mybir.dt.float32
    f16 = mybir.dt.float16
    Exp = mybir.ActivationFunctionType.Exp
    Mult = mybir.AluOpType.mult

    nc = bacc.Bacc("TRN2", target_bir_lowering=False, debug=False,
                   num_devices=NCORES)

    XT = nc.dram_tensor("xt", [D, S], f16, kind="ExternalInput").ap()
    WQ = nc.dram_tensor("wq", [D + 1, COLS], f16, kind="ExternalInput").ap()
    WK = nc.dram_tensor("wk", [D + 1, COLS], f16, kind="ExternalInput").ap()
    WV = nc.dram_tensor("wv", [D + 1, VCOLS], f16, kind="ExternalInput").ap()
    WO = nc.dram_tensor("wo", [COLS, D], f16, kind="ExternalInput").ap()
    OT = nc.dram_tensor("ot", [D, S], f16, kind="ExternalOutput").ap()

    def packed_dma(dst_tile, src, rows_per_chunk, chunk_w, nchunk):
        """One DMA: DRAM [nchunk*rows, chunk_w] -> SBUF [rows, nchunk*chunk_w]."""
        dst_ap = bass.AP(tensor=dst_tile.tensor, offset=dst_tile.offset,
                         ap=[dst_tile.ap[0], [chunk_w, nchunk], [1, chunk_w]])
        src_ap = bass.AP(tensor=src.tensor, offset=src.offset,
                         ap=[[chunk_w, rows_per_chunk],
                             [rows_per_chunk * chunk_w, nchunk], [1, chunk_w]])
        nc.sync.dma_start(dst_ap, src_ap)

    with tile.TileContext(nc) as tc, ExitStack() as ctx:
        consts = ctx.enter_context(tc.tile_pool(name="consts", bufs=1))
        drp = ctx.enter_context(tc.tile_pool(name="drp", bufs=2, space="DRAM"))

        # ---- weights (one DMA each; wk first — K proj is critical
        # path) and X chunks on the gpsimd queue in parallel ----
        # dummy weight-load source for PE warmup/fillers
        dummy_w = consts.tile([128, 128], f16, name="dummy_w")
        nc.vector.memset(dummy_w, 0.0)

        def filler(n):
            """Dummy weight loads: keep the PE activity monitor from
            re-throttling the core during PE-idle stretches (the core
            drops to half clock otherwise, ~20% wall)."""
            for _ in range(n):
                nc.tensor.ldweights(dummy_w)

        # warm the PE while the launch-phase DMA queues boot (~9us):
        # the HAM starts cold (1.2GHz) and needs ~3.4us of activity
        filler(52)

        wk_all = consts.tile([128, ND * COLS], f16, name="wk_all")
        packed_dma(wk_all, WK, 128, COLS, ND)

        xc = []
        for d in range(ND):
            t = consts.tile([128, S], f16, name=f"xc_{d}")
            eng = nc.gpsimd if d % 2 == 0 else nc.sync
            eng.dma_start(t, XT[d * 128:(d + 1) * 128, :])
            xc.append(t)

        wv_all = consts.tile([128, ND * VCOLS], f16, name="wv_all")
        packed_dma(wv_all, WV, 128, VCOLS, ND)
        wq_all = consts.tile([128, ND * COLS], f16, name="wq_all")
        packed_dma(wq_all, WQ, 128, COLS, ND)
        wo_all = consts.tile([128, 2 * D], f16, name="wo_all")
        packed_dma(wo_all, WO, 128, D, 2)

        # per-partition bias columns for the K/Q drains
        def bias_col(name, src_row, ct):
            t16 = consts.tile([128, 1], f16, name=name + "h")
            dst = bass.AP(tensor=t16.tensor, offset=t16.offset,
                          ap=[t16.ap[0], [1, 1]])
            s = src_row[0:1, ct * 128:(ct + 1) * 128]
            src_ap = bass.AP(tensor=s.tensor, offset=s.offset,
                             ap=[[1, 128], [1, 1]])
            nc.sync.dma_start(dst, src_ap)
            t = consts.tile([128, 1], f32, name=name)
            nc.vector.tensor_copy(t, t16)
            return t

        wkb = [bias_col(f"wkb_{ct}", WK[D:D + 1, :], ct) for ct in range(2)]
        wqb = [bias_col(f"wqb_{ct}", WQ[D:D + 1, :], ct) for ct in range(2)]
        wv_b = consts.tile([1, VCOLS], f16, name="wv_b")
        nc.sync.dma_start(wv_b, WV[D:D + 1, :])

        ones_row = consts.tile([1, S], f16, name="ones_row")
        nc.vector.memset(ones_row, 1.0)
        onecol_16 = consts.tile([1, DK], f16, name="onecol")
        nc.vector.memset(onecol_16, 1.0)

        KT = [consts.tile([128, S], f16, name=f"KT_{ct}") for ct in range(2)]
        QT = [consts.tile([128, S], f16, name=f"QT_{ct}") for ct in range(2)]
        VP = [consts.tile([128, VCOLS], f16, name=f"VP_{kt}")
              for kt in range(NKT)]

        def wk_c(d, cs):
            return wk_all[:, d * COLS + cs.start:d * COLS + cs.stop]

        def wq_c(d, cs):
            return wq_all[:, d * COLS + cs.start:d * COLS + cs.stop]

        def wv_c(d):
            return wv_all[:, d * VCOLS:(d + 1) * VCOLS]

        # ---- phase 1: K projection d-outer (consumes X chunks as they
        # arrive), then V' tiles 0..NVA-1 d-outer on the same banks ----
        with tc.tile_pool(name="pk", bufs=8, space="PSUM") as pk:
            kps = [pk.tile([128, 512], f32, tag="k", name=f"kps_{i}")
                   for i in range(8)]
            for d in range(ND):
                for ct in range(2):
                    cs = slice(ct * 128, (ct + 1) * 128)
                    for tch in range(NT):
                        ts = slice(tch * 512, (tch + 1) * 512)
                        nc.tensor.matmul(kps[ct * NT + tch],
                                         lhsT=wk_c(d, cs), rhs=xc[d][:, ts],
                                         start=(d == 0), stop=(d == ND - 1))
            for ct in range(2):
                for tch in range(NT):
                    ts = slice(tch * 512, (tch + 1) * 512)
                    if (ct * NT + tch) % 2:
                        nc.scalar.activation(
                            KT[ct][:, ts], kps[ct * NT + tch],
                            mybir.ActivationFunctionType.Identity,
                            bias=wkb[ct])
                    else:
                        nc.vector.tensor_scalar_add(KT[ct][:, ts],
                                                    kps[ct * NT + tch],
                                                    wkb[ct])
            filler(12)
            vps = [pk.tile([128, VCOLS], f32, tag="k", name=f"vps_{i}")
                   for i in range(NVA)]
            for d in range(ND + 1):
                for kt in range(NVA):
                    ks = slice(kt * 128, (kt + 1) * 128)
                    if d < ND:
                        lhsT, rhs = xc[d][:, ks], wv_c(d)
                    else:
                        lhsT, rhs = ones_row[:, ks], wv_b
                    nc.tensor.matmul(vps[kt], lhsT=lhsT, rhs=rhs,
                                     start=(d == 0), stop=(d == ND))
            for kt in range(NVA):
                if kt % 2:
                    nc.scalar.copy(VP[kt], vps[kt])
                else:
                    nc.vector.tensor_copy(VP[kt], vps[kt])

        # ---- phase 2: flat attention + drip-fed projections ----
        with tc.tile_pool(name="ps", bufs=2, space="PSUM") as ps, \
                tc.tile_pool(name="expp", bufs=14) as expp, \
                tc.tile_pool(name="attnp", bufs=4) as attnp, \
                tc.tile_pool(name="zp", bufs=2) as zp, \
                tc.tile_pool(name="zbp", bufs=2) as zbp, \
                tc.tile_pool(name="obp", bufs=3) as obp:

            segs = [(qc, pr) for qc in range(NQC) for pr in range(NPR)]
            attn_t = {}

            def sc_pair(qc, pr, kt, ndum=0):
                """Packed score pair: head A on PE rows 0-63, head B on
                rows 64-127 — concurrent via row tiling.  ndum small
                dummy matmuls precede them (immediately overwritten):
                LDWEIGHTS alone seems invisible to the HAM activity
                monitor, so idle top-up must be real matmul fill."""
                qs = slice(qc * 512, (qc + 1) * 512)
                ks = slice(kt * 128, (kt + 1) * 128)
                sc = ps.tile([128, 1024], f32, tag="sc", bufs=2,
                             name=f"sc_{qc}_{pr}_{kt}")
                for i in range(ndum):
                    nc.tensor.matmul(sc[:, 0:128], lhsT=dummy_w,
                                     rhs=dummy_w, start=True, stop=True,
                                     skip_group_check=True)
                nc.tensor.matmul(sc[:, 0:512], lhsT=KT[pr][0:64, ks],
                                 rhs=QT[pr][0:64, qs], start=True, stop=True)
                nc.tensor.matmul(sc[:, 512:1024], lhsT=KT[pr][64:128, ks],
                                 rhs=QT[pr][64:128, qs], start=True,
                                 stop=True)
                return sc

            def q_chain_jobs(pr, tch):
                """Project one Q quadrant; 8 single-matmul jobs + a
                bias-folding drain, dripped one per kt. The PSUM chain
                slot is allocated lazily by the first job so the
                tag-ring rotation matches emission order."""
                cs = slice(pr * 128, (pr + 1) * 128)
                ts = slice(tch * 512, (tch + 1) * 512)
                st = {}

                def mk(d):
                    def job():
                        if d == 0:
                            st["p"] = ps.tile([128, 512], f32, tag="c",
                                              bufs=2, name=f"qch_{pr}_{tch}")
                        nc.tensor.matmul(st["p"], lhsT=wq_c(d, cs),
                                         rhs=xc[d][:, ts],
                                         start=(d == 0), stop=(d == ND - 1))
                    return job

                def cast():
                    nc.vector.tensor_scalar_add(QT[pr][:, ts], st["p"],
                                                wqb[pr])

                return [mk(d) for d in range(ND)] + [cast]

            def q_chain(pr, tch):
                for job in q_chain_jobs(pr, tch):
                    job()

            def vp_chain(kt, cast_eng="scalar"):
                """Project one V' token-tile into a chain slot."""
                ks = slice(kt * 128, (kt + 1) * 128)
                p = ps.tile([128, VCOLS], f32, tag="c", bufs=2,
                            name=f"vch_{kt}")
                for d in range(ND + 1):
                    if d < ND:
                        lhsT, rhs = xc[d][:, ks], wv_c(d)
                    else:
                        lhsT, rhs = ones_row[:, ks], wv_b
                    nc.tensor.matmul(p, lhsT=lhsT, rhs=rhs,
                                     start=(d == 0), stop=(d == ND))
                if cast_eng == "scalar":
                    nc.scalar.copy(VP[kt], p)
                else:
                    nc.vector.tensor_copy(VP[kt], p)

            def outproj_e(qc, e, tail=False):
                es = slice(e * 128, (e + 1) * 128)
                po = ps.tile([128, 512], f32, tag="c", bufs=2,
                             name=f"po_{qc}_{e}")
                for cc in range(2):
                    nc.tensor.matmul(
                        po, lhsT=wo_all[:, cc * D + es.start:cc * D + es.stop],
                        rhs=attn_t[(qc, cc)], start=(cc == 0), stop=(cc == 1))
                ob = obp.tile([128, 512], f16, tag="ob", name=f"ob_{qc}_{e}")
                if tail:
                    # ACT is idle at the tail — split the drain
                    nc.vector.tensor_copy(ob[:, 0:256], po[:, 0:256])
                    nc.scalar.copy(ob[:, 256:512], po[:, 256:512])
                else:
                    nc.vector.tensor_copy(ob, po)
                qbase = qc * 512
                eng = nc.sync if e % 2 == 0 else nc.gpsimd
                eng.dma_start(OT[es, qbase:qbase + 512], ob)

            def finish_seg(patt, qc, pr, tail):
                """Normalize the accumulated pair: z rows -> reciprocal
                -> partition broadcast -> multiply into attn_t."""
                at = attnp.tile([128, 512], f16, tag="attn",
                                name=f"attn_{qc}_{pr}")
                attn_t[(qc, pr)] = at
                zs = zp.tile([1, 1024], f32, tag="zs", name=f"zs_{qc}_{pr}")
                nc.vector.tensor_copy(zs, patt[DK:DK + 1, :])
                zr = zp.tile([1, 1024], f32, tag="z", name=f"zr_{qc}_{pr}")
                nc.vector.reciprocal_approx_fast(zr, zs)
                if tail:
                    filler(30)
                    # last segment: broadcast 1/z via PE outer products
                    # (short latency; the DRAM bounce would gate the tail)
                    zr16 = zp.tile([1, 1024], f16, tag="z16",
                                   name=f"zr16_{qc}_{pr}")
                    nc.vector.tensor_copy(zr16, zr)
                    araw = zbp.tile([DK, 1024], f32, tag="zb",
                                    name=f"araw_{qc}_{pr}")
                    nc.scalar.copy(araw, patt[0:DK, :])
                    zba = ps.tile([64, 512], f32, tag="c", bufs=2,
                                  name=f"zba_{qc}_{pr}")
                    nc.tensor.matmul(zba, lhsT=onecol_16[:, 0:DK],
                                     rhs=zr16[:, 0:512], start=True,
                                     stop=True)
                    zbb = ps.tile([64, 512], f32, tag="c", bufs=2,
                                  name=f"zbb_{qc}_{pr}")
                    nc.tensor.matmul(zbb, lhsT=onecol_16[:, 0:DK],
                                     rhs=zr16[:, 512:1024], start=True,
                                     stop=True)
                    nc.vector.tensor_tensor(at[0:DK, :], araw[:, 0:512],
                                            zba, Mult)
                    nc.vector.tensor_tensor(at[DK:128, :],
                                            araw[:, 512:1024], zbb, Mult)
                else:
                    # broadcast 1/z across partitions via a DRAM bounce
                    # (partition-stride-0 DMA read)
                    scratch = drp.tile([1, 1024], f32, tag="scr",
                                       name=f"scr_{qc}_{pr}")
                    nc.gpsimd.dma_start(scratch, zr)
                    zb = zbp.tile([DK, 1024], f32, tag="zb",
                                  name=f"zb_{qc}_{pr}")
                    row = scratch[0:1, :]
                    nc.gpsimd.dma_start(
                        zb,
                        bass.AP(tensor=row.tensor, offset=row.offset,
                                ap=[[0, DK]] + row.ap[1:]))
                    nc.vector.tensor_tensor(at[0:DK, :], patt[0:DK, 0:512],
                                            zb[:, 0:512], Mult)
                    nc.vector.tensor_tensor(at[DK:128, :],
                                            patt[0:DK, 512:1024],
                                            zb[:, 512:1024], Mult)
                filler(4)

            # drip plan: (si, kt) -> list of (cost_ns, thunk).  Each kt
            # is topped up with dummy weight-loads to a uniform PE load
            # slightly above the ACT exp pace: a deliberately PE-bound
            # loop stays at full clock (HAM) and tolerates jitter,
            # whereas a balanced one oscillates against the ring limits.
            drip = {}

            def add_drip(si, kt, job, cost=230):
                drip.setdefault((si, kt), []).append((cost, job))

            # s0: the p1 Q quadrant early (segment 1 needs it), then V'
            # JIT for tiles NVA..15 with a 4-kt lead
            qj = q_chain_jobs(1, 0)
            for j, job in enumerate(qj):
                add_drip(0, j * 3 // 9, job)       # kts 0..3
            for j, kt in enumerate(range(NVA, NKT)):
                add_drip(0, kt - 4, (lambda k, e: lambda: vp_chain(k, e))(
                    kt, "scalar" if j % 2 else "vector"), cost=1200)
            # q-chain segments: segment 1 -> tch 1, 3 -> tch 2, 5 -> tch 3
            for si, tch in ((1, 1), (3, 2), (5, 3)):
                for pr in range(2):
                    jobs = q_chain_jobs(pr, tch)
                    for j, job in enumerate(jobs):
                        add_drip(si, min(j + 7 * pr, NKT - 1), job)
            # outproj segments: qc finished after segment 2*qc+1; its 8
            # column chunks drip over the following segment (one per
            # odd kt)
            for qc in range(NQC - 1):
                for e in range(NE):
                    add_drip(2 * qc + 2, 2 * e + 1,
                             (lambda q, ee: lambda: outproj_e(q, ee))(qc, e),
                             cost=520)

            def make_attnv(patt, kt, ex, vs_a, vs_b):
                def emit():
                    nc.tensor.matmul(patt[:, 0:512], lhsT=VP[kt][:, vs_a],
                                     rhs=ex[:, 0:512], start=(kt == 0),
                                     stop=(kt == NKT - 1))
                    nc.tensor.matmul(patt[:, 512:1024],
                                     lhsT=VP[kt][:, vs_b],
                                     rhs=ex[:, 512:1024], start=(kt == 0),
                                     stop=(kt == NKT - 1))
                return emit

            # pre-loop: first Q quadrant, then the first score pair.
            # attnV runs one kt BEHIND the exp/score stream so the score
            # matmuls are never queued behind an exp-waiting attnV
            # (head-of-line blocking starves ACT and trips the HAM
            # throttle).
            q_chain(0, 0)
            sc = sc_pair(0, 0, 0)
            pend = None
            pend_fin = None
            for si, (qc, pr) in enumerate(segs):
                patt = ps.tile([DK + 1, 1024], f32, tag="acc", bufs=1,
                               name=f"patt_{qc}_{pr}")
                vs_a = slice((2 * pr) * (DK + 1), (2 * pr) * (DK + 1) + DK + 1)
                vs_b = slice((2 * pr + 1) * (DK + 1),
                             (2 * pr + 1) * (DK + 1) + DK + 1)
                for kt in range(NKT):
                    ex = expp.tile([128, 1024], f16, tag="e",
                                   name=f"ex_{qc}_{pr}_{kt}")
                    nc.scalar.activation(ex, sc, Exp, scale=0.125)
                    # prefetch the next score pair (crossing boundaries)
                    nd = 0 if (si, kt) in drip else 2
                    if kt + 1 < NKT:
                        sc = sc_pair(qc, pr, kt + 1, ndum=nd)
                    elif si + 1 < len(segs):
                        sc = sc_pair(segs[si + 1][0], segs[si + 1][1], 0,
                                     ndum=nd)
                    if pend is not None:
                        pend()
                    if pend_fin is not None:
                        # previous segment's last attnV just went out;
                        # its normalization must precede this patt's
                        # first write (the accumulator is 1-deep)
                        pend_fin()
                        pend_fin = None
                    for c, job in drip.pop((si, kt), ()):
                        job()
                    pend = make_attnv(patt, kt, ex, vs_a, vs_b)
                pend_fin = (lambda p, q, r: lambda: finish_seg(
                    p, q, r, tail=False))(patt, qc, pr)
            pend()
            finish_seg(patt, NQC - 1, NPR - 1, tail=True)
            # flush the last query chunk's output projection
            for e in range(NE):
                outproj_e(NQC - 1, e, tail=True)

    nc.compile()
    return nc


def _get_program():
    if "nc" not in _CACHE:
        _CACHE["nc"] = _build_program()
    return _CACHE["nc"]


def _install_ntff_hook():
    """Provide the antenv.axon_hooks shim this container's antenv lacks so
    run_bass_kernel_spmd(trace=True) can capture NTFF profiles."""
    import sys
    import types

    try:
        import antenv

        if hasattr(antenv, "axon_hooks"):
            return
        mod = types.ModuleType("antenv.axon_hooks")
        mod._hook = None
        mod.set_axon_ntff_profile_hook = lambda h: setattr(mod, "_hook", h)
        mod.get_axon_ntff_profile_hook = lambda: mod._hook
        sys.modules["antenv.axon_hooks"] = mod
        antenv.axon_hooks = mod
        from trn_agent_boot.trn_boot import _ntff_profile_via_ctypes

        mod.set_axon_ntff_profile_hook(
            _ntff_profile_via_ctypes("/opt/axon/libaxon_pjrt.so"))
    except Exception:
        pass


def kernel(X, Wq, bq, Wk, bk, Wv, bv, Wo, bo, _profile=False, _trace_cores=None):
    from concourse.bass_utils import run_bass_kernel_spmd

    if _profile:
        _install_ntff_hook()

    nc = _get_program()

    X = np.asarray(X, np.float32)
    Wq, Wk, Wv, Wo = (np.asarray(w, np.float32) for w in (Wq, Wk, Wv, Wo))
    bq, bk, bv, bo = (np.asarray(v, np.float32) for v in (bq, bk, bv, bo))

    in_maps = []
    for c in range(NCORES):
        b, g = c // GPB, c % GPB
        cs = slice(g * COLS, (g + 1) * COLS)
        wq_aug = np.vstack([Wq[:, cs], bq[cs][None, :]])
        wk_aug = np.vstack([Wk[:, cs], bk[cs][None, :]])
        wv_aug = np.zeros((D + 1, VCOLS), np.float32)
        for h in range(HPC):
            hc = slice(g * COLS + h * DK, g * COLS + (h + 1) * DK)
            wv_aug[:D, h * (DK + 1):h * (DK + 1) + DK] = Wv[:, hc]
            wv_aug[D, h * (DK + 1):h * (DK + 1) + DK] = bv[hc]
            wv_aug[D, h * (DK + 1) + DK] = 1.0
        in_maps.append({
            "xt": np.ascontiguousarray(X[b].T).astype(np.float16),
            "wq": wq_aug.astype(np.float16),
            "wk": wk_aug.astype(np.float16),
            "wv": wv_aug.astype(np.float16),
            "wo": np.ascontiguousarray(Wo[cs, :]).astype(np.float16),
        })

    res = run_bass_kernel_spmd(
        nc, in_maps, core_ids=list(range(NCORES)),
        trace=_profile,
        trace_cores=(_trace_cores if _trace_cores is not None
                     else ([0] if _profile else None)),
    )

    out = np.empty((B, S, D), np.float32)
    for b in range(B):
        ot = res.results[b * GPB]["ot"].astype(np.float64)
        for g in range(1, GPB):
            ot += res.results[b * GPB + g]["ot"]
        out[b] = (ot.T + bo).astype(np.float32)
    if _profile:
        kernel.last_exec_time_ns = res.exec_time_ns
        kernel.last_results = res
    return out


# revision 27
# speedup vs baseline: 1.1593x; 1.1245x over previous
"""Multi-head self-attention on 8 TRN2 NeuronCores.

Sharding: (batch, head-quad). Core c owns batch c//4 and heads
4*(c%4)..4*(c%4)+3, i.e. a 256-column slice of Wq/Wk/Wv and the matching
256-row slice of Wo. The host sums the 4 partial outputs per batch (the
tensor-parallel all-reduce) and adds the output bias.

Structure (v3): the attention inner loop runs on 512-query chunks and
processes a HEAD PAIR per segment so the two score matmuls (contraction
dk=64) land on different PE row-groups (tile_position (0,0)/(64,0)) and
execute concurrently — the packed pair costs one matmul slot instead of
two. The exp of both heads' scores is a single 1024-wide activation from
one 2-bank PSUM tile; ACT paces the loop and all projection / output
work drips into the PE slack through a dedicated 2-slot PSUM chain ring,
so the score ring never loses lookahead.

- K projection runs d-outer across 8 PSUM banks to pipeline with X DMA;
  V' for key tiles 0-7 reuses those banks d-outer right after; V' tiles
  8-15 drip as chains inside segment 0.
- Q/K biases are folded into the PSUM->SBUF drain (per-partition bias on
  ACT / tensor_scalar_add on DVE) instead of K=1 matmuls.  V' keeps the
  ones-row matmul because its bias varies along the free dim (and it
  also provides the softmax-normalizer column).
- Dummy LDWEIGHTS fill PE idle slots: the HAM activity monitor
  re-throttles the whole core to half clock when the PE duty cycle sags
  (costs ~20% wall clock), so staying busy is faster than idling.
- softmax 1/z is broadcast across partitions via a DRAM bounce
  (partition-stride-0 DMA read); gpsimd Q7 compute also downclocks.
- one dma_start per weight matrix (descriptor building on the queue
  engines is ~0.6us each and delays the X stream otherwise).

Precision: f16 activations/weights on the PE; PSUM f32; partial outputs
f16 summed in f64 on host.
"""

import numpy as np

B, S, D, H, DK = 2, 2048, 1024, 16, 64
NCORES = 8
GPB = 4                    # head-groups per batch
HPC = H // GPB             # heads per core = 4
COLS = HPC * DK            # feature columns per core = 256
VCOLS = HPC * (DK + 1)     # V' columns incl per-head ones = 260
ND = D // 128              # 8 contraction chunks
NT = S // 512              # 4 token chunks (free dim 512)
NKT = S // 128             # 16 key tiles
NQC = S // 512             # 4 query chunks
NPR = 2                    # head pairs per core
NE = D // 128              # 8 output-column chunks
NVA = 8                    # V' tiles projected d-outer before attention

_CACHE = {}


def _build_program():
    from contextlib import ExitStack

    import concourse.bacc as bacc
    import concourse.bass as bass
    import concourse.mybir as mybir
    import concourse.tile as tile

    f32 = mybir.dt.float32
    f16 = mybir.dt.float16
    Exp = mybir.ActivationFunctionType.Exp
    Mult = mybir.AluOpType.mult

    nc = bacc.Bacc("TRN2", target_bir_lowering=False, debug=False,
                   num_devices=NCORES)

    XT = nc.dram_tensor("xt", [D, S], f16, kind="ExternalInput").ap()
    WQ = nc.dram_tensor("wq", [D + 1, COLS], f16, kind="ExternalInput").ap()
    WK = nc.dram_tensor("wk", [D + 1, COLS], f16, kind="ExternalInput").ap()
    WV = nc.dram_tensor("wv", [D + 1, VCOLS], f16, kind="ExternalInput").ap()
    WO = nc.dram_tensor("wo", [COLS, D], f16, kind="ExternalInput").ap()
    OT = nc.dram_tensor("ot", [D, S], f16, kind="ExternalOutput").ap()

    def packed_dma(dst_tile, src, rows_per_chunk, chunk_w, nchunk):
        """One DMA: DRAM [nchunk*rows, chunk_w] -> SBUF [rows, nchunk*chunk_w]."""
        dst_ap = bass.AP(tensor=dst_tile.tensor, offset=dst_tile.offset,
                         ap=[dst_tile.ap[0], [chunk_w, nchunk], [1, chunk_w]])
        src_ap = bass.AP(tensor=src.tensor, offset=src.offset,
                         ap=[[chunk_w, rows_per_chunk],
                             [rows_per_chunk * chunk_w, nchunk], [1, chunk_w]])
        nc.sync.dma_start(dst_ap, src_ap)

    with tile.TileContext(nc) as tc, ExitStack() as ctx:
        consts = ctx.enter_context(tc.tile_pool(name="consts", bufs=1))
        drp = ctx.enter_context(tc.tile_pool(name="drp", bufs=2, space="DRAM"))

        # ---- weights (one DMA each; wk first — K proj is critical
        # path) and X chunks on the gpsimd queue in parallel ----
        # dummy weight-load source for PE warmup/fillers
        dummy_w = consts.tile([128, 128], f16, name="dummy_w")
        nc.vector.memset(dummy_w, 0.0)

        def filler(n):
            """Dummy weight loads: keep the PE activity monitor from
            re-throttling the core during PE-idle stretches (the core
            drops to half clock otherwise, ~20% wall)."""
            for _ in range(n):
                nc.tensor.ldweights(dummy_w)

        # warm the PE while the launch-phase DMA queues boot (~9us):
        # the HAM starts cold (1.2GHz) and needs ~3.4us of activity
        filler(52)

        wk_all = consts.tile([128, ND * COLS], f16, name="wk_all")
        packed_dma(wk_all, WK, 128, COLS, ND)

        xc = []
        for d in range(ND):
            t = consts.tile([128, S], f16, name=f"xc_{d}")
            eng = nc.gpsimd if d % 2 == 0 else nc.sync
            eng.dma_start(t, XT[d * 128:(d + 1) * 128, :])
            xc.append(t)

        wv_all = consts.tile([128, ND * VCOLS], f16, name="wv_all")
        packed_dma(wv_all, WV, 128, VCOLS, ND)
        wq_all = consts.tile([128, ND * COLS], f16, name="wq_all")
        packed_dma(wq_all, WQ, 128, COLS, ND)
        wo_all = consts.tile([128, 2 * D], f16, name="wo_all")
        packed_dma(wo_all, WO, 128, D, 2)

        # per-partition bias columns for the K/Q drains
        def bias_col(name, src_row, ct):
            t16 = consts.tile([128, 1], f16, name=name + "h")
            dst = bass.AP(tensor=t16.tensor, offset=t16.offset,
                          ap=[t16.ap[0], [1, 1]])
            s = src_row[0:1, ct * 128:(ct + 1) * 128]
            src_ap = bass.AP(tensor=s.tensor, offset=s.offset,
                             ap=[[1, 128], [1, 1]])
            nc.sync.dma_start(dst, src_ap)
            t = consts.tile([128, 1], f32, name=name)
            nc.vector.tensor_copy(t, t16)
            return t

        wkb = [bias_col(f"wkb_{ct}", WK[D:D + 1, :], ct) for ct in range(2)]
        wqb = [bias_col(f"wqb_{ct}", WQ[D:D + 1, :], ct) for ct in range(2)]
        wv_b = consts.tile([1, VCOLS], f16, name="wv_b")
        nc.sync.dma_start(wv_b, WV[D:D + 1, :])

        ones_row = consts.tile([1, S], f16, name="ones_row")
        nc.vector.memset(ones_row, 1.0)
        onecol_16 = consts.tile([1, DK], f16, name="onecol")
        nc.vector.memset(onecol_16, 1.0)

        KT = [consts.tile([128, S], f16, name=f"KT_{ct}") for ct in range(2)]
        QT = [consts.tile([128, S], f16, name=f"QT_{ct}") for ct in range(2)]
        VP = [consts.tile([128, VCOLS], f16, name=f"VP_{kt}")
              for kt in range(NKT)]

        def wk_c(d, cs):
            return wk_all[:, d * COLS + cs.start:d * COLS + cs.stop]

        def wq_c(d, cs):
            return wq_all[:, d * COLS + cs.start:d * COLS + cs.stop]

        def wv_c(d):
            return wv_all[:, d * VCOLS:(d + 1) * VCOLS]

        # ---- phase 1: K projection d-outer (consumes X chunks as they
        # arrive), then V' tiles 0..NVA-1 d-outer on the same banks ----
        with tc.tile_pool(name="pk", bufs=8, space="PSUM") as pk:
            kps = [pk.tile([128, 512], f32, tag="k", name=f"kps_{i}")
                   for i in range(8)]
            for d in range(ND):
                for ct in range(2):
                    cs = slice(ct * 128, (ct + 1) * 128)
                    for tch in range(NT):
                        ts = slice(tch * 512, (tch + 1) * 512)
                        nc.tensor.matmul(kps[ct * NT + tch],
                                         lhsT=wk_c(d, cs), rhs=xc[d][:, ts],
                                         start=(d == 0), stop=(d == ND - 1))
            for ct in range(2):
                for tch in range(NT):
                    ts = slice(tch * 512, (tch + 1) * 512)
                    if (ct * NT + tch) % 2:
                        nc.scalar.activation(
                            KT[ct][:, ts], kps[ct * NT + tch],
                            mybir.ActivationFunctionType.Identity,
                            bias=wkb[ct])
                    else:
                        nc.vector.tensor_scalar_add(KT[ct][:, ts],
                                                    kps[ct * NT + tch],
                                                    wkb[ct])
            filler(12)
            vps = [pk.tile([128, VCOLS], f32, tag="k", name=f"vps_{i}")
                   for i in range(NVA)]
            for d in range(ND + 1):
                for kt in range(NVA):
                    ks = slice(kt * 128, (kt + 1) * 128)
                    if d < ND:
                        lhsT, rhs = xc[d][:, ks], wv_c(d)
                    else:
                        lhsT, rhs = ones_row[:, ks], wv_b
                    nc.tensor.matmul(vps[kt], lhsT=lhsT, rhs=rhs,
                                     start=(d == 0), stop=(d == ND))
            for kt in range(NVA):
                if kt % 2:
                    nc.scalar.copy(VP[kt], vps[kt])
                else:
                    nc.vector.tensor_copy(VP[kt], vps[kt])

        # ---- phase 2: flat attention + drip-fed projections ----
        with tc.tile_pool(name="ps", bufs=2, space="PSUM") as ps, \
                tc.tile_pool(name="expp", bufs=14) as expp, \
                tc.tile_pool(name="attnp", bufs=4) as attnp, \
                tc.tile_pool(name="zp", bufs=2) as zp, \
                tc.tile_pool(name="zbp", bufs=2) as zbp, \
                tc.tile_pool(name="obp", bufs=3) as obp:

            segs = [(qc, pr) for qc in range(NQC) for pr in range(NPR)]
            attn_t = {}

            def sc_pair(qc, pr, kt, ndum=0):
                """Packed score pair: head A on PE rows 0-63, head B on
                rows 64-127 — concurrent via row tiling.  ndum small
                dummy matmuls precede them (immediately overwritten):
                LDWEIGHTS alone seems invisible to the HAM activity
                monitor, so idle top-up must be real matmul fill."""
                qs = slice(qc * 512, (qc + 1) * 512)
                ks = slice(kt * 128, (kt + 1) * 128)
                sc = ps.tile([128, 1024], f32, tag="sc", bufs=2,
                             name=f"sc_{qc}_{pr}_{kt}")
                for i in range(ndum):
                    nc.tensor.matmul(sc[:, 0:128], lhsT=dummy_w,
                                     rhs=dummy_w, start=True, stop=True,
                                     skip_group_check=True)
                nc.tensor.matmul(sc[:, 0:512], lhsT=KT[pr][0:64, ks],
                                 rhs=QT[pr][0:64, qs], start=True, stop=True)
                nc.tensor.matmul(sc[:, 512:1024], lhsT=KT[pr][64:128, ks],
                                 rhs=QT[pr][64:128, qs], start=True,
                                 stop=True)
                return sc

            def q_chain_jobs(pr, tch):
                """Project one Q quadrant; 8 single-matmul jobs + a
                bias-folding drain, dripped one per kt. The PSUM chain
                slot is allocated lazily by the first job so the
                tag-ring rotation matches emission order."""
                cs = slice(pr * 128, (pr + 1) * 128)
                ts = slice(tch * 512, (tch + 1) * 512)
                st = {}

                def mk(d):
                    def job():
                        if d == 0:
                            st["p"] = ps.tile([128, 512], f32, tag="c",
                                              bufs=2, name=f"qch_{pr}_{tch}")
                        nc.tensor.matmul(st["p"], lhsT=wq_c(d, cs),
                                         rhs=xc[d][:, ts],
                                         start=(d == 0), stop=(d == ND - 1))
                    return job

                def cast():
                    nc.vector.tensor_scalar_add(QT[pr][:, ts], st["p"],
                                                wqb[pr])

                return [mk(d) for d in range(ND)] + [cast]

            def q_chain(pr, tch):
                for job in q_chain_jobs(pr, tch):
                    job()

            def vp_chain(kt, cast_eng="scalar"):
                """Project one V' token-tile into a chain slot."""
                ks = slice(kt * 128, (kt + 1) * 128)
                p = ps.tile([128, VCOLS], f32, tag="c", bufs=2,
                            name=f"vch_{kt}")
                for d in range(ND + 1):
                    if d < ND:
                        lhsT, rhs = xc[d][:, ks], wv_c(d)
                    else:
                        lhsT, rhs = ones_row[:, ks], wv_b
                    nc.tensor.matmul(p, lhsT=lhsT, rhs=rhs,
                                     start=(d == 0), stop=(d == ND))
                if cast_eng == "scalar":
                    nc.scalar.copy(VP[kt], p)
                else:
                    nc.vector.tensor_copy(VP[kt], p)

            def outproj_e(qc, e, tail=False):
                es = slice(e * 128, (e + 1) * 128)
                po = ps.tile([128, 512], f32, tag="c", bufs=2,
                             name=f"po_{qc}_{e}")
                for cc in range(2):
                    nc.tensor.matmul(
                        po, lhsT=wo_all[:, cc * D + es.start:cc * D + es.stop],
                        rhs=attn_t[(qc, cc)], start=(cc == 0), stop=(cc == 1))
                ob = obp.tile([128, 512], f16, tag="ob", name=f"ob_{qc}_{e}")
                if tail:
                    # ACT is idle at the tail — split the drain
                    nc.vector.tensor_copy(ob[:, 0:256], po[:, 0:256])
                    nc.scalar.copy(ob[:, 256:512], po[:, 256:512])
                else:
                    nc.vector.tensor_copy(ob, po)
                qbase = qc * 512
                eng = nc.sync if e % 2 == 0 else nc.gpsimd
                eng.dma_start(OT[es, qbase:qbase + 512], ob)

            def finish_seg(patt, qc, pr, tail):
                """Normalize the accumulated pair.  The accumulator is
                1-deep, so free it FAST: one DVE copy drains all 65 rows
                (DVE cost is free-size only); reciprocal, broadcast and
                the multiplies then run off the SBUF copy while the next
                segment accumulates."""
                at = attnp.tile([128, 512], f16, tag="attn",
                                name=f"attn_{qc}_{pr}")
                attn_t[(qc, pr)] = at
                araw = zbp.tile([DK, 1024], f32, tag="ar",
                                name=f"araw_{qc}_{pr}")
                nc.vector.tensor_copy(araw, patt[0:DK, :])
                if tail:
                    zs = zp.tile([1, 1024], f32, tag="zs",
                                 name=f"zs_{qc}_{pr}")
                    nc.vector.tensor_copy(zs, patt[DK:DK + 1, :])
                    zr = zp.tile([1, 1024], f32, tag="z",
                                 name=f"zr_{qc}_{pr}")
                    nc.vector.reciprocal_approx_fast(zr, zs)
                    # last segment: broadcast 1/z via PE outer products
                    # (short latency; the DRAM bounce would gate the tail)
                    zr16 = zp.tile([1, 1024], f16, tag="z16",
                                   name=f"zr16_{qc}_{pr}")
                    nc.vector.tensor_copy(zr16, zr)
                    zba = ps.tile([64, 512], f32, tag="c", bufs=2,
                                  name=f"zba_{qc}_{pr}")
                    nc.tensor.matmul(zba, lhsT=onecol_16[:, 0:DK],
                                     rhs=zr16[:, 0:512], start=True,
                                     stop=True)
                    zbb = ps.tile([64, 512], f32, tag="c", bufs=2,
                                  name=f"zbb_{qc}_{pr}")
                    nc.tensor.matmul(zbb, lhsT=onecol_16[:, 0:DK],
                                     rhs=zr16[:, 512:1024], start=True,
                                     stop=True)
                    nc.vector.tensor_tensor(at[0:DK, :], araw[:, 0:512],
                                            zba, Mult)
                    nc.vector.tensor_tensor(at[DK:128, :],
                                            araw[:, 512:1024], zbb, Mult)
                else:
                    # broadcast raw z across partitions via a DRAM
                    # bounce straight out of PSUM (frees the accumulator
                    # without a DVE pass), then take the reciprocal on
                    # the broadcast tile
                    zs = zp.tile([1, 1024], f32, tag="zs",
                                 name=f"zs_{qc}_{pr}")
                    nc.scalar.copy(zs, patt[DK:DK + 1, :])
                    scratch = drp.tile([1, 1024], f32, tag="scr",
                                       name=f"scr_{qc}_{pr}")
                    nc.gpsimd.dma_start(scratch, zs)
                    zbraw = zbp.tile([DK, 1024], f32, tag="zb",
                                     name=f"zbraw_{qc}_{pr}")
                    row = scratch[0:1, :]
                    nc.gpsimd.dma_start(
                        zbraw,
                        bass.AP(tensor=row.tensor, offset=row.offset,
                                ap=[[0, DK]] + row.ap[1:]))
                    zbr = zbp.tile([DK, 1024], f32, tag="zr",
                                   name=f"zbr_{qc}_{pr}")
                    nc.vector.reciprocal_approx_fast(zbr, zbraw)
                    nc.vector.tensor_tensor(at[0:DK, :], araw[:, 0:512],
                                            zbr[:, 0:512], Mult)
                    nc.vector.tensor_tensor(at[DK:128, :],
                                            araw[:, 512:1024],
                                            zbr[:, 512:1024], Mult)
                filler(4)

            # drip plan: (si, kt) -> list of (cost_ns, thunk).  Each kt
            # is topped up with dummy weight-loads to a uniform PE load
            # slightly above the ACT exp pace: a deliberately PE-bound
            # loop stays at full clock (HAM) and tolerates jitter,
            # whereas a balanced one oscillates against the ring limits.
            drip = {}

            def add_drip(si, kt, job, cost=230):
                drip.setdefault((si, kt), []).append((cost, job))

            # s0: the p1 Q quadrant early (segment 1 needs it), then V'
            # JIT for tiles NVA..15 with a 4-kt lead
            qj = q_chain_jobs(1, 0)
            for j, job in enumerate(qj):
                add_drip(0, j * 3 // 9, job)       # kts 0..3
            for j, kt in enumerate(range(NVA, NKT)):
                add_drip(0, kt - 4, (lambda k, e: lambda: vp_chain(k, e))(
                    kt, "scalar" if j % 2 else "vector"), cost=1200)
            # q-chain segments: segment 1 -> tch 1, 3 -> tch 2, 5 -> tch 3
            for si, tch in ((1, 1), (3, 2), (5, 3)):
                for pr in range(2):
                    jobs = q_chain_jobs(pr, tch)
                    for j, job in enumerate(jobs):
                        add_drip(si, min(j + 7 * pr, NKT - 1), job)
            # outproj segments: qc finished after segment 2*qc+1; its 8
            # column chunks drip over the following segment (one per
            # odd kt)
            for qc in range(NQC - 1):
                for e in range(NE):
                    add_drip(2 * qc + 2, 2 * e + 1,
                             (lambda q, ee: lambda: outproj_e(q, ee))(qc, e),
                             cost=520)

            def make_attnv(patt, kt, ex, vs_a, vs_b):
                def emit():
                    nc.tensor.matmul(patt[:, 0:512], lhsT=VP[kt][:, vs_a],
                                     rhs=ex[:, 0:512], start=(kt == 0),
                                     stop=(kt == NKT - 1))
                    nc.tensor.matmul(patt[:, 512:1024],
                                     lhsT=VP[kt][:, vs_b],
                                     rhs=ex[:, 512:1024], start=(kt == 0),
                                     stop=(kt == NKT - 1))
                return emit

            # pre-loop: first Q quadrant, then the first score pair.
            # attnV runs one kt BEHIND the exp/score stream so the score
            # matmuls are never queued behind an exp-waiting attnV
            # (head-of-line blocking starves ACT and trips the HAM
            # throttle).
            q_chain(0, 0)
            sc = sc_pair(0, 0, 0)
            pend = None
            pend_fin = None
            for si, (qc, pr) in enumerate(segs):
                patt = ps.tile([DK + 1, 1024], f32, tag="acc", bufs=1,
                               name=f"patt_{qc}_{pr}")
                vs_a = slice((2 * pr) * (DK + 1), (2 * pr) * (DK + 1) + DK + 1)
                vs_b = slice((2 * pr + 1) * (DK + 1),
                             (2 * pr + 1) * (DK + 1) + DK + 1)
                for kt in range(NKT):
                    ex = expp.tile([128, 1024], f16, tag="e",
                                   name=f"ex_{qc}_{pr}_{kt}")
                    nc.scalar.activation(ex, sc, Exp, scale=0.125)
                    # prefetch the next score pair (crossing boundaries)
                    nd = 0
                    if kt + 1 < NKT:
                        sc = sc_pair(qc, pr, kt + 1, ndum=nd)
                    elif si + 1 < len(segs):
                        sc = sc_pair(segs[si + 1][0], segs[si + 1][1], 0,
                                     ndum=nd)
                    if pend is not None:
                        pend()
                    if pend_fin is not None:
                        # previous segment's last attnV just went out;
                        # its normalization must precede this patt's
                        # first write (the accumulator is 1-deep)
                        pend_fin()
                        pend_fin = None
                    for c, job in drip.pop((si, kt), ()):
                        job()
                    pend = make_attnv(patt, kt, ex, vs_a, vs_b)
                pend_fin = (lambda p, q, r: lambda: finish_seg(
                    p, q, r, tail=False))(patt, qc, pr)
            pend()
            finish_seg(patt, NQC - 1, NPR - 1, tail=True)
            # flush the last query chunk's output projection
            for e in range(NE):
                outproj_e(NQC - 1, e, tail=True)

    nc.compile()
    return nc


def _get_program():
    if "nc" not in _CACHE:
        _CACHE["nc"] = _build_program()
    return _CACHE["nc"]


def _install_ntff_hook():
    """Provide the antenv.axon_hooks shim this container's antenv lacks so
    run_bass_kernel_spmd(trace=True) can capture NTFF profiles."""
    import sys
    import types

    try:
        import antenv

        if hasattr(antenv, "axon_hooks"):
            return
        mod = types.ModuleType("antenv.axon_hooks")
        mod._hook = None
        mod.set_axon_ntff_profile_hook = lambda h: setattr(mod, "_hook", h)
        mod.get_axon_ntff_profile_hook = lambda: mod._hook
        sys.modules["antenv.axon_hooks"] = mod
        antenv.axon_hooks = mod
        from trn_agent_boot.trn_boot import _ntff_profile_via_ctypes

        mod.set_axon_ntff_profile_hook(
            _ntff_profile_via_ctypes("/opt/axon/libaxon_pjrt.so"))
    except Exception:
        pass


def kernel(X, Wq, bq, Wk, bk, Wv, bv, Wo, bo, _profile=False, _trace_cores=None):
    from concourse.bass_utils import run_bass_kernel_spmd

    if _profile:
        _install_ntff_hook()

    nc = _get_program()

    X = np.asarray(X, np.float32)
    Wq, Wk, Wv, Wo = (np.asarray(w, np.float32) for w in (Wq, Wk, Wv, Wo))
    bq, bk, bv, bo = (np.asarray(v, np.float32) for v in (bq, bk, bv, bo))

    in_maps = []
    for c in range(NCORES):
        b, g = c // GPB, c % GPB
        cs = slice(g * COLS, (g + 1) * COLS)
        wq_aug = np.vstack([Wq[:, cs], bq[cs][None, :]])
        wk_aug = np.vstack([Wk[:, cs], bk[cs][None, :]])
        wv_aug = np.zeros((D + 1, VCOLS), np.float32)
        for h in range(HPC):
            hc = slice(g * COLS + h * DK, g * COLS + (h + 1) * DK)
            wv_aug[:D, h * (DK + 1):h * (DK + 1) + DK] = Wv[:, hc]
            wv_aug[D, h * (DK + 1):h * (DK + 1) + DK] = bv[hc]
            wv_aug[D, h * (DK + 1) + DK] = 1.0
        in_maps.append({
            "xt": np.ascontiguousarray(X[b].T).astype(np.float16),
            "wq": wq_aug.astype(np.float16),
            "wk": wk_aug.astype(np.float16),
            "wv": wv_aug.astype(np.float16),
            "wo": np.ascontiguousarray(Wo[cs, :]).astype(np.float16),
        })

    res = run_bass_kernel_spmd(
        nc, in_maps, core_ids=list(range(NCORES)),
        trace=_profile,
        trace_cores=(_trace_cores if _trace_cores is not None
                     else ([0] if _profile else None)),
    )

    out = np.empty((B, S, D), np.float32)
    for b in range(B):
        ot = res.results[b * GPB]["ot"].astype(np.float64)
        for g in range(1, GPB):
            ot += res.results[b * GPB + g]["ot"]
        out[b] = (ot.T + bo).astype(np.float32)
    if _profile:
        kernel.last_exec_time_ns = res.exec_time_ns
        kernel.last_results = res
    return out
